# revision 7
# baseline (speedup 1.0000x reference)
"""Trainium2 Bass kernel for nn_CrossAttnQKVModule.

Strategy
--------
Head-parallel over 8 NeuronCores: head h owns the stride-8 slice [h::8] of
every weight output-channel dim (channel layout in the module is
c = hid * NUM_HEADS + head, so head h's channels are c % 8 == h).

The Pin-equivariant linear (equi_linear) collapses, per head, into ONE dense
matmul: the 9-map basis contraction is folded into the weights on the host,
giving a combined matrix W [K=640, N=320] where
  K = 512 multivector features (i_ch * 16 + blade) + 128 scalar features
  N = 320 output cols = [mv block (8ch x 16 blades) | 32 s] x 2 selections
Gathers (q_idx / kv_idx item selection) commute with the per-item linear, so
we gather input rows first and compute directly in output order:
  pass A rows = q_idx  -> outputs [q_mv | q_s | v_mv_q | v_s_q]
  pass B rows = kv_idx -> outputs [k_mv | k_s | v_mv   | v_s  ]

Device pipeline per 128-item tile: indirect-DMA row gather (chunked, 1024
rows/op) -> 5 PE transposes (128x128) -> PSUM->SBUF copies -> 5 accumulating
float32r matmuls (K=640 in 5 chunks, N=320) -> bias add -> HWDGE store.
"""

import contextlib
import ctypes
import os
import sys
import types

import numpy as np

import concourse.bacc as bacc
import concourse.bass as bass
import concourse.mybir as mybir
import concourse.tile as tile
from concourse import library_config
from concourse.bass_utils import run_bass_kernel_spmd
from concourse.masks import make_identity
from concourse.tile import add_dep_helper


def _ensure_axon_ntff_hook():
    """bass_utils imports antenv.axon_hooks when trace=True under axon; the
    container ships only a stub antenv package. Inject a working hook module
    backed by direct ctypes calls into libaxon_pjrt.so (same ABI the boot
    shim uses) so NTFF profiling works."""
    try:
        import antenv.axon_hooks  # noqa: F401
        return
    except ImportError:
        pass
    mod = types.ModuleType("antenv.axon_hooks")
    _state = {"hook": None}

    so_path = "/opt/axon/libaxon_pjrt.so"
    if os.path.exists(so_path):
        try:
            lib = ctypes.CDLL(so_path)
            if hasattr(lib, "axon_start_nrt_profile"):
                lib.axon_start_nrt_profile.argtypes = [
                    ctypes.POINTER(ctypes.c_int64), ctypes.c_size_t]
                lib.axon_start_nrt_profile.restype = ctypes.c_int64
                lib.axon_stop_nrt_profile.argtypes = [ctypes.c_char_p]
                lib.axon_stop_nrt_profile.restype = ctypes.c_int64

                @contextlib.contextmanager
                def _hook(output_dir, device_ids):
                    import jax
                    jax.devices()
                    if device_ids:
                        ids = (ctypes.c_int64 * len(device_ids))(*device_ids)
                        rc = lib.axon_start_nrt_profile(ids, len(device_ids))
                    else:
                        rc = lib.axon_start_nrt_profile(None, 0)
                    if rc != 0:
                        raise RuntimeError(f"axon_start_nrt_profile rc={rc}")
                    try:
                        yield
                    finally:
                        n = lib.axon_stop_nrt_profile(str(output_dir).encode())
                        print(f"ntff profile: {n} file(s) -> {output_dir}",
                              file=sys.stderr)

                _state["hook"] = _hook
        except OSError:
            pass

    mod.get_axon_ntff_profile_hook = lambda: _state["hook"]
    mod.set_axon_ntff_profile_hook = lambda h: _state.update(hook=h)
    sys.modules["antenv.axon_hooks"] = mod


_ensure_axon_ntff_hook()

# ---- problem constants (hardcoded; kernel.py must be self-contained)
B, N = 2, 4096
M = B * N                 # 8192 flattened items
C_MV, C_S = 32, 128
NUM_HEADS, HID_MV, HID_S = 8, 8, 32
NF = C_MV * 16 + C_S      # 640 input features per item
NOUT = 320                # output cols per pass: 128 mv + 32 s, x2 selections
P = 128
TILES = M // P            # 64 item-tiles per pass
CHUNK = 8                 # item-tiles gathered per indirect DMA
NCHUNK = TILES // CHUNK
NCORES = 8

_CACHE = {}
LAST_RESULTS = None       # test.py reads this for profiling info


def _build_basis() -> np.ndarray:
    """(9, 16, 16) basis[y, x_out, a_in]: 5 grade projections + 4 e0-left-mult."""
    grades = [[0], [1, 2, 3, 4], [5, 6, 7, 8, 9, 10], [11, 12, 13, 14], [15]]
    maps = []
    for g in grades:
        m = np.zeros((16, 16), np.float32)
        m[g, g] = 1.0
        maps.append(m)
    e0_pairs = [[(1, 0)], [(5, 2), (6, 3), (7, 4)], [(11, 8), (12, 9), (13, 10)],
                [(15, 14)]]
    for pairs in e0_pairs:
        m = np.zeros((16, 16), np.float32)
        for dst, src in pairs:
            m[dst, src] = 1.0
        maps.append(m)
    return np.stack(maps)


_BASIS = _build_basis()


def _fill_block(W, b, col0, w_mv, w_s2mv, w_mv2s, w_s2s, b_s):
    """Fill one 160-col block (128 mv + 32 s) of the combined weight matrix.

    w_mv: (8, C_MV, 9), w_s2mv: (8, C_S), w_mv2s: (32, C_MV), w_s2s: (32, C_S),
    b_s: (32,). Feature rows: f = i*16 + a for mv, 512 + j for scalars.
    mv col = col0 + ch*16 + x; s col = col0 + 128 + cs.
    """
    # out_mv[ch, x] += sum_{i,a,y} w_mv[ch,i,y] * BASIS[y,x,a] * in[i,a]
    wmv = np.einsum("oiy,yxa->iaox", w_mv, _BASIS).reshape(512, 128)
    W[0:512, col0:col0 + 128] = wmv
    # out_mv[ch, 0] += s2mv @ scalars
    W[512:640, col0:col0 + 128:16] = w_s2mv.T
    # out_s[cs] += mv2s @ in[:, 0]  (blade 0 features are rows i*16)
    W[0:512:16, col0 + 128:col0 + 160] = w_mv2s.T
    # out_s[cs] += s2s @ scalars
    W[512:640, col0 + 128:col0 + 160] = w_s2s.T
    b[col0 + 128:col0 + 160] = b_s


def _build_core_weights(h, q_w_mv, q_w_s2mv, q_w_mv2s, q_w_s2s, q_b_s,
                        kv_w_mv, kv_w_s2mv, kv_w_mv2s, kv_w_s2s, kv_b_s):
    """Combined per-head weights: W [2, 640, 320], bias [2, 320]."""
    W = np.zeros((2, NF, NOUT), np.float32)
    bvec = np.zeros((2, NOUT), np.float32)
    # pass A: q block (cols 0:160) + v block (cols 160:320)
    _fill_block(W[0], bvec[0], 0,
                q_w_mv[h::8], q_w_s2mv[h::8], q_w_mv2s[h::8], q_w_s2s[h::8],
                q_b_s[h::8])
    _fill_block(W[0], bvec[0], 160,
                kv_w_mv[64 + h::8], kv_w_s2mv[64 + h::8],
                kv_w_mv2s[256 + h::8], kv_w_s2s[256 + h::8], kv_b_s[256 + h::8])
    # pass B: k block (cols 0:160) + v block (cols 160:320)
    _fill_block(W[1], bvec[1], 0,
                kv_w_mv[h:64:8], kv_w_s2mv[h:64:8],
                kv_w_mv2s[h:256:8], kv_w_s2s[h:256:8], kv_b_s[h:256:8])
    _fill_block(W[1], bvec[1], 160,
                kv_w_mv[64 + h::8], kv_w_s2mv[64 + h::8],
                kv_w_mv2s[256 + h::8], kv_w_s2s[256 + h::8], kv_b_s[256 + h::8])
    return W, bvec


def build_module():
    """Build + compile the SPMD Bass module (same program on all 8 cores)."""
    if "nc" in _CACHE:
        return _CACHE["nc"]
    nc = bacc.Bacc("TRN2", target_bir_lowering=False, debug=False,
                   num_devices=NCORES)
    f32 = mybir.dt.float32
    f32r = mybir.dt.float32r
    x_d = nc.dram_tensor("x", [M, 512], f32, kind="ExternalInput")
    s_d = nc.dram_tensor("s", [M, 128], f32, kind="ExternalInput")
    w_d = nc.dram_tensor("w", [2, NF, NOUT], f32r, kind="ExternalInput")
    b_d = nc.dram_tensor("bias", [2, P, NOUT], f32, kind="ExternalInput")
    i_d = nc.dram_tensor("idx", [2, NCHUNK, P, CHUNK * 8], mybir.dt.int16,
                         kind="ExternalInput")
    o_d = nc.dram_tensor("out", [2, M, NOUT], f32, kind="ExternalOutput")

    with tile.TileContext(nc) as tc:
        with (
            tc.tile_pool(name="singles", bufs=1) as singles,
            tc.tile_pool(name="chunks", bufs=3) as chunks,
            tc.tile_pool(name="xtp", bufs=3) as xtp,
            tc.tile_pool(name="outs", bufs=4) as outs,
            tc.tile_pool(name="psumT", bufs=2, space="PSUM") as psumT_pool,
            tc.tile_pool(name="psumO", bufs=4, space="PSUM") as psumO_pool,
        ):
            lib = nc.gpsimd.load_library(library_config.mlp)
            ident = singles.tile([P, P], f32)
            make_identity(nc, ident)
            w_sb = singles.tile([P, 2, 5, NOUT], f32r)
            nc.sync.dma_start(
                out=w_sb, in_=w_d.ap().rearrange("s (c p) n -> p s c n", p=P))
            bias_sb = singles.tile([P, 2, NOUT], f32)
            nc.sync.dma_start(
                out=bias_sb, in_=b_d.ap().rearrange("s p n -> p s n"))
            idx_sb = singles.tile([P, 2, NCHUNK, CHUNK * 8], mybir.dt.int16)
            nc.sync.dma_start(
                out=idx_sb, in_=i_d.ap().rearrange("s c p t -> p s c t"))

            NIDX = CHUNK * P
            for sidx in range(2):
                for c in range(NCHUNK):
                    cx = chunks.tile([P, CHUNK, 512], f32, tag="cx")
                    cs_ = chunks.tile([P, CHUNK, 128], f32, tag="cs")
                    off = idx_sb[:, sidx, c, :]
                    g1 = nc.gpsimd.dma_gather(
                        cx[:, :, :], x_d.ap(), off, NIDX, NIDX, 512)
                    g2 = nc.gpsimd.dma_gather(
                        cs_[:, :, :], s_d.ap(), off, NIDX, NIDX, 128)
                    add_dep_helper(g1.ins, lib.ins, reason="lib before gather")
                    add_dep_helper(g2.ins, lib.ins, reason="lib before gather")
                    for tt in range(CHUNK):
                        t = c * CHUNK + tt
                        pT = psumT_pool.tile([P, NF], f32)
                        for kc in range(4):
                            nc.tensor.transpose(
                                out=pT[:, kc * 128:(kc + 1) * 128],
                                in_=cx[:, tt, kc * 128:(kc + 1) * 128],
                                identity=ident)
                        nc.tensor.transpose(
                            out=pT[:, 512:640], in_=cs_[:, tt, :],
                            identity=ident)
                        xT = xtp.tile([P, NF], f32r)
                        for kc in range(5):
                            nc.any.tensor_copy(
                                out=xT[:, kc * 128:(kc + 1) * 128],
                                in_=pT[:, kc * 128:(kc + 1) * 128])
                        po = psumO_pool.tile([P, NOUT], f32)
                        for kc in range(5):
                            nc.tensor.matmul(
                                out=po[:, :],
                                lhsT=xT[:, kc * 128:(kc + 1) * 128],
                                rhs=w_sb[:, sidx, kc, :],
                                start=(kc == 0), stop=(kc == 4))
                        ob = outs.tile([P, NOUT], f32)
                        nc.vector.tensor_add(
                            out=ob[:, :], in0=po[:, :],
                            in1=bias_sb[:, sidx, :])
                        nc.sync.dma_start(
                            out=o_d.ap()[sidx, t * P:(t + 1) * P, :],
                            in_=ob[:, :])
    nc.compile()
    _CACHE["nc"] = nc
    return nc


def _prep_inputs(inputs, scalars, q_idx, kv_idx, wargs):
    """Host-side shard prep: returns per-core in_maps."""
    x_flat = np.ascontiguousarray(
        np.asarray(inputs, dtype=np.float32).reshape(M, 512))
    s_flat = np.ascontiguousarray(
        np.asarray(scalars, dtype=np.float32).reshape(M, 128))

    q_idx = np.asarray(q_idx).astype(np.int64)
    kv_idx = np.asarray(kv_idx).astype(np.int64)
    boff = (np.arange(B, dtype=np.int64) * N)[:, None]
    gidx = np.stack([(boff + q_idx[None, :]).reshape(-1),
                     (boff + kv_idx[None, :]).reshape(-1)])  # [2, M]
    # dma_gather idx layout per 1024-row chunk: index j = s*16 + p sits at
    # [p, s], replicated across the 8 16-partition groups (one per Q7 core)
    NIDX = CHUNK * P
    idx_dev = np.empty((2, NCHUNK, P, NIDX // 16), np.int16)
    for sidx in range(2):
        for c in range(NCHUNK):
            flat = gidx[sidx, c * NIDX:(c + 1) * NIDX].astype(np.int16)
            idx_dev[sidx, c] = np.tile(flat.reshape(NIDX // 16, 16).T, (8, 1))
    idx_dev = np.ascontiguousarray(idx_dev)

    in_maps = []
    for h in range(NCORES):
        W, bvec = _build_core_weights(h, *wargs)
        bias_bcast = np.ascontiguousarray(
            np.broadcast_to(bvec[:, None, :], (2, P, NOUT))).astype(np.float32)
        in_maps.append({
            "x": x_flat, "s": s_flat,
            "w": np.ascontiguousarray(W),
            "bias": bias_bcast, "idx": idx_dev,
        })
    return in_maps


def kernel(inputs, scalars, q_w_mv, q_w_s2mv, q_w_mv2s, q_w_s2s, q_b_s,
           kv_w_mv, kv_w_s2mv, kv_w_mv2s, kv_w_s2s, kv_b_s, q_idx, kv_idx):
    global LAST_RESULTS
    nc = build_module()
    wargs = tuple(np.asarray(a, dtype=np.float32) for a in (
        q_w_mv, q_w_s2mv, q_w_mv2s, q_w_s2s, q_b_s,
        kv_w_mv, kv_w_s2mv, kv_w_mv2s, kv_w_s2s, kv_b_s))
    in_maps = _prep_inputs(inputs, scalars, q_idx, kv_idx, wargs)
    res = run_bass_kernel_spmd(nc, in_maps, core_ids=list(range(NCORES)))
    LAST_RESULTS = res
    o = np.stack([r["out"] for r in res.results])  # [8, 2, M, 320]
    A, Bp = o[:, 0], o[:, 1]

    def mv(block, c0):
        return np.ascontiguousarray(
            block[:, :, c0:c0 + 128].reshape(NCORES, B, N, HID_MV, 16)
            .transpose(1, 0, 2, 3, 4))

    def sc(block, c0):
        return np.ascontiguousarray(
            block[:, :, c0:c0 + 32].reshape(NCORES, B, N, HID_S)
            .transpose(1, 0, 2, 3))

    q_mv, q_s = mv(A, 0), sc(A, 128)
    v_mv_queries, v_s_queries = mv(A, 160), sc(A, 288)
    k_mv, k_s = mv(Bp, 0), sc(Bp, 128)
    v_mv, v_s = mv(Bp, 160), sc(Bp, 288)
    return (q_mv, k_mv, v_mv, v_mv_queries, q_s, k_s, v_s, v_s_queries)


# revision 10
# speedup vs baseline: 1.4134x; 1.4134x over previous
"""Trainium2 Bass kernel for nn_CrossAttnQKVModule.

Strategy
--------
Head-parallel over 8 NeuronCores: head h owns the stride-8 slice [h::8] of
every weight output-channel dim (channel layout in the module is
c = hid * NUM_HEADS + head, so head h's channels are c % 8 == h).

The Pin-equivariant linear (equi_linear) collapses, per head, into ONE dense
matmul: the 9-map basis contraction is folded into the weights on the host,
giving a combined matrix W [K=640, N=320] where
  K = 512 multivector features (i_ch * 16 + blade) + 128 scalar features
  N = 320 output cols = [mv block (8ch x 16 blades) | 32 s] x 2 selections
Gathers (q_idx / kv_idx item selection) commute with the per-item linear, so
we gather input rows first and compute directly in output order:
  pass A rows = q_idx  -> outputs [q_mv | q_s | v_mv_q | v_s_q]
  pass B rows = kv_idx -> outputs [k_mv | k_s | v_mv   | v_s  ]

Device pipeline per 128-item tile: indirect-DMA row gather (chunked, 1024
rows/op) -> 5 PE transposes (128x128) -> PSUM->SBUF copies -> 5 accumulating
float32r matmuls (K=640 in 5 chunks, N=320) -> bias add -> HWDGE store.
"""

import contextlib
import ctypes
import os
import sys
import types

import numpy as np

import concourse.bacc as bacc
import concourse.bass as bass
import concourse.mybir as mybir
import concourse.tile as tile
from concourse import library_config
from concourse.bass_utils import run_bass_kernel_spmd
from concourse.masks import make_identity
from concourse.tile import add_dep_helper


def _ensure_axon_ntff_hook():
    """bass_utils imports antenv.axon_hooks when trace=True under axon; the
    container ships only a stub antenv package. Inject a working hook module
    backed by direct ctypes calls into libaxon_pjrt.so (same ABI the boot
    shim uses) so NTFF profiling works."""
    try:
        import antenv.axon_hooks  # noqa: F401
        return
    except ImportError:
        pass
    mod = types.ModuleType("antenv.axon_hooks")
    _state = {"hook": None}

    so_path = "/opt/axon/libaxon_pjrt.so"
    if os.path.exists(so_path):
        try:
            lib = ctypes.CDLL(so_path)
            if hasattr(lib, "axon_start_nrt_profile"):
                lib.axon_start_nrt_profile.argtypes = [
                    ctypes.POINTER(ctypes.c_int64), ctypes.c_size_t]
                lib.axon_start_nrt_profile.restype = ctypes.c_int64
                lib.axon_stop_nrt_profile.argtypes = [ctypes.c_char_p]
                lib.axon_stop_nrt_profile.restype = ctypes.c_int64

                @contextlib.contextmanager
                def _hook(output_dir, device_ids):
                    import jax
                    jax.devices()
                    if device_ids:
                        ids = (ctypes.c_int64 * len(device_ids))(*device_ids)
                        rc = lib.axon_start_nrt_profile(ids, len(device_ids))
                    else:
                        rc = lib.axon_start_nrt_profile(None, 0)
                    if rc != 0:
                        raise RuntimeError(f"axon_start_nrt_profile rc={rc}")
                    try:
                        yield
                    finally:
                        n = lib.axon_stop_nrt_profile(str(output_dir).encode())
                        print(f"ntff profile: {n} file(s) -> {output_dir}",
                              file=sys.stderr)

                _state["hook"] = _hook
        except OSError:
            pass

    mod.get_axon_ntff_profile_hook = lambda: _state["hook"]
    mod.set_axon_ntff_profile_hook = lambda h: _state.update(hook=h)
    sys.modules["antenv.axon_hooks"] = mod


_ensure_axon_ntff_hook()

# ---- problem constants (hardcoded; kernel.py must be self-contained)
B, N = 2, 4096
M = B * N                 # 8192 flattened items
C_MV, C_S = 32, 128
NUM_HEADS, HID_MV, HID_S = 8, 8, 32
NF = C_MV * 16 + C_S      # 640 input features per item
NOUT = 320                # output cols per pass: 128 mv + 32 s, x2 selections
P = 128
TILES = M // P            # 64 item-tiles per pass
CHUNK = 8                 # item-tiles gathered per indirect DMA
NCHUNK = TILES // CHUNK
NCORES = 8

_CACHE = {}
LAST_RESULTS = None       # test.py reads this for profiling info


def _build_basis() -> np.ndarray:
    """(9, 16, 16) basis[y, x_out, a_in]: 5 grade projections + 4 e0-left-mult."""
    grades = [[0], [1, 2, 3, 4], [5, 6, 7, 8, 9, 10], [11, 12, 13, 14], [15]]
    maps = []
    for g in grades:
        m = np.zeros((16, 16), np.float32)
        m[g, g] = 1.0
        maps.append(m)
    e0_pairs = [[(1, 0)], [(5, 2), (6, 3), (7, 4)], [(11, 8), (12, 9), (13, 10)],
                [(15, 14)]]
    for pairs in e0_pairs:
        m = np.zeros((16, 16), np.float32)
        for dst, src in pairs:
            m[dst, src] = 1.0
        maps.append(m)
    return np.stack(maps)


_BASIS = _build_basis()


def _fill_block(W, b, col0, w_mv, w_s2mv, w_mv2s, w_s2s, b_s):
    """Fill one 160-col block (128 mv + 32 s) of the combined weight matrix.

    w_mv: (8, C_MV, 9), w_s2mv: (8, C_S), w_mv2s: (32, C_MV), w_s2s: (32, C_S),
    b_s: (32,). Feature rows: f = i*16 + a for mv, 512 + j for scalars.
    mv col = col0 + ch*16 + x; s col = col0 + 128 + cs.
    """
    # out_mv[ch, x] += sum_{i,a,y} w_mv[ch,i,y] * BASIS[y,x,a] * in[i,a]
    wmv = np.einsum("oiy,yxa->iaox", w_mv, _BASIS).reshape(512, 128)
    W[0:512, col0:col0 + 128] = wmv
    # out_mv[ch, 0] += s2mv @ scalars
    W[512:640, col0:col0 + 128:16] = w_s2mv.T
    # out_s[cs] += mv2s @ in[:, 0]  (blade 0 features are rows i*16)
    W[0:512:16, col0 + 128:col0 + 160] = w_mv2s.T
    # out_s[cs] += s2s @ scalars
    W[512:640, col0 + 128:col0 + 160] = w_s2s.T
    b[col0 + 128:col0 + 160] = b_s


def _build_core_weights(h, q_w_mv, q_w_s2mv, q_w_mv2s, q_w_s2s, q_b_s,
                        kv_w_mv, kv_w_s2mv, kv_w_mv2s, kv_w_s2s, kv_b_s):
    """Combined per-head weights: W [2, 640, 320], bias [2, 320]."""
    W = np.zeros((2, NF, NOUT), np.float32)
    bvec = np.zeros((2, NOUT), np.float32)
    # pass A: q block (cols 0:160) + v block (cols 160:320)
    _fill_block(W[0], bvec[0], 0,
                q_w_mv[h::8], q_w_s2mv[h::8], q_w_mv2s[h::8], q_w_s2s[h::8],
                q_b_s[h::8])
    _fill_block(W[0], bvec[0], 160,
                kv_w_mv[64 + h::8], kv_w_s2mv[64 + h::8],
                kv_w_mv2s[256 + h::8], kv_w_s2s[256 + h::8], kv_b_s[256 + h::8])
    # pass B: k block (cols 0:160) + v block (cols 160:320)
    _fill_block(W[1], bvec[1], 0,
                kv_w_mv[h:64:8], kv_w_s2mv[h:64:8],
                kv_w_mv2s[h:256:8], kv_w_s2s[h:256:8], kv_b_s[h:256:8])
    _fill_block(W[1], bvec[1], 160,
                kv_w_mv[64 + h::8], kv_w_s2mv[64 + h::8],
                kv_w_mv2s[256 + h::8], kv_w_s2s[256 + h::8], kv_b_s[256 + h::8])
    return W, bvec


def build_module():
    """Build + compile the SPMD Bass module (same program on all 8 cores)."""
    if "nc" in _CACHE:
        return _CACHE["nc"]
    nc = bacc.Bacc("TRN2", target_bir_lowering=False, debug=False,
                   num_devices=NCORES)
    f32 = mybir.dt.float32
    f16 = mybir.dt.float16
    x_d = nc.dram_tensor("x16", [M, NF], f16, kind="ExternalInput")
    w_d = nc.dram_tensor("w", [2, NF, NOUT], f16, kind="ExternalInput")
    b_d = nc.dram_tensor("bias", [2, P, NOUT], f32, kind="ExternalInput")
    i_d = nc.dram_tensor("idx", [2, NCHUNK, P, CHUNK * 8], mybir.dt.int16,
                         kind="ExternalInput")
    o_d = nc.dram_tensor("out", [2, M, NOUT], f32, kind="ExternalOutput")

    NIDX = CHUNK * P
    with tile.TileContext(nc) as tc:
        with (
            tc.tile_pool(name="singles", bufs=1) as singles,
            tc.tile_pool(name="chunks", bufs=3) as chunks,
            tc.tile_pool(name="outs", bufs=4) as outs,
            tc.tile_pool(name="psumO", bufs=6, space="PSUM") as psumO_pool,
        ):
            lib = nc.gpsimd.load_library(library_config.mlp)
            w_sb = singles.tile([P, 2, 5, NOUT], f16)
            nc.sync.dma_start(
                out=w_sb, in_=w_d.ap().rearrange("s (c p) n -> p s c n", p=P))
            bias_sb = singles.tile([P, 2, NOUT], f32)
            nc.sync.dma_start(
                out=bias_sb, in_=b_d.ap().rearrange("s p n -> p s n"))
            idx_sb = singles.tile([P, 2, NCHUNK, CHUNK * 8], mybir.dt.int16)
            nc.sync.dma_start(
                out=idx_sb, in_=i_d.ap().rearrange("s c p t -> p s c t"))

            for sidx in range(2):
                for c in range(NCHUNK):
                    # transposed gather: xT[p, kc, i] = x16[idx[i], kc*128+p]
                    xT = chunks.tile([P, 5, NIDX], f16, tag="xT")
                    g1 = nc.gpsimd.dma_gather(
                        xT[:, :, :], x_d.ap(), idx_sb[:, sidx, c, :],
                        NIDX, NIDX, NF, transpose=True, single_packet=False)
                    add_dep_helper(g1.ins, lib.ins, reason="lib before gather")
                    for tt in range(CHUNK):
                        t = c * CHUNK + tt
                        po = psumO_pool.tile([P, NOUT], f32)
                        for kc in range(5):
                            nc.tensor.matmul(
                                out=po[:, :],
                                lhsT=xT[:, kc, tt * P:(tt + 1) * P],
                                rhs=w_sb[:, sidx, kc, :],
                                start=(kc == 0), stop=(kc == 4))
                        ob = outs.tile([P, NOUT], f32)
                        nc.vector.tensor_add(
                            out=ob[:, :], in0=po[:, :],
                            in1=bias_sb[:, sidx, :])
                        nc.sync.dma_start(
                            out=o_d.ap()[sidx, t * P:(t + 1) * P, :],
                            in_=ob[:, :])
    nc.compile()
    _CACHE["nc"] = nc
    return nc


def _prep_inputs(inputs, scalars, q_idx, kv_idx, wargs):
    """Host-side shard prep: returns per-core in_maps."""
    x16 = np.empty((M, NF), np.float16)
    x16[:, :512] = np.asarray(inputs, dtype=np.float32).reshape(M, 512)
    x16[:, 512:] = np.asarray(scalars, dtype=np.float32).reshape(M, 128)

    q_idx = np.asarray(q_idx).astype(np.int64)
    kv_idx = np.asarray(kv_idx).astype(np.int64)
    boff = (np.arange(B, dtype=np.int64) * N)[:, None]
    gidx = np.stack([(boff + q_idx[None, :]).reshape(-1),
                     (boff + kv_idx[None, :]).reshape(-1)])  # [2, M]
    # dma_gather idx layout per 1024-row chunk: index j = s*16 + p sits at
    # [p, s], replicated across the 8 16-partition groups (one per Q7 core)
    NIDX = CHUNK * P
    idx_dev = np.empty((2, NCHUNK, P, NIDX // 16), np.int16)
    for sidx in range(2):
        for c in range(NCHUNK):
            flat = gidx[sidx, c * NIDX:(c + 1) * NIDX].astype(np.int16)
            idx_dev[sidx, c] = np.tile(flat.reshape(NIDX // 16, 16).T, (8, 1))
    idx_dev = np.ascontiguousarray(idx_dev)

    in_maps = []
    for h in range(NCORES):
        W, bvec = _build_core_weights(h, *wargs)
        bias_bcast = np.ascontiguousarray(
            np.broadcast_to(bvec[:, None, :], (2, P, NOUT))).astype(np.float32)
        in_maps.append({
            "x16": x16,
            "w": np.ascontiguousarray(W.astype(np.float16)),
            "bias": bias_bcast, "idx": idx_dev,
        })
    return in_maps


def kernel(inputs, scalars, q_w_mv, q_w_s2mv, q_w_mv2s, q_w_s2s, q_b_s,
           kv_w_mv, kv_w_s2mv, kv_w_mv2s, kv_w_s2s, kv_b_s, q_idx, kv_idx):
    global LAST_RESULTS
    nc = build_module()
    wargs = tuple(np.asarray(a, dtype=np.float32) for a in (
        q_w_mv, q_w_s2mv, q_w_mv2s, q_w_s2s, q_b_s,
        kv_w_mv, kv_w_s2mv, kv_w_mv2s, kv_w_s2s, kv_b_s))
    in_maps = _prep_inputs(inputs, scalars, q_idx, kv_idx, wargs)
    res = run_bass_kernel_spmd(nc, in_maps, core_ids=list(range(NCORES)))
    LAST_RESULTS = res
    o = np.stack([r["out"] for r in res.results])  # [8, 2, M, 320]
    A, Bp = o[:, 0], o[:, 1]

    def mv(block, c0):
        return np.ascontiguousarray(
            block[:, :, c0:c0 + 128].reshape(NCORES, B, N, HID_MV, 16)
            .transpose(1, 0, 2, 3, 4))

    def sc(block, c0):
        return np.ascontiguousarray(
            block[:, :, c0:c0 + 32].reshape(NCORES, B, N, HID_S)
            .transpose(1, 0, 2, 3))

    q_mv, q_s = mv(A, 0), sc(A, 128)
    v_mv_queries, v_s_queries = mv(A, 160), sc(A, 288)
    k_mv, k_s = mv(Bp, 0), sc(Bp, 128)
    v_mv, v_s = mv(Bp, 160), sc(Bp, 288)
    return (q_mv, k_mv, v_mv, v_mv_queries, q_s, k_s, v_s, v_s_queries)


# revision 12
# speedup vs baseline: 1.5812x; 1.1187x over previous
"""Trainium2 Bass kernel for nn_CrossAttnQKVModule.

Strategy
--------
Head-parallel over 8 NeuronCores: head h owns the stride-8 slice [h::8] of
every weight output-channel dim (channel layout in the module is
c = hid * NUM_HEADS + head, so head h's channels are c % 8 == h).

The Pin-equivariant linear (equi_linear) collapses, per head, into ONE dense
matmul: the 9-map basis contraction is folded into the weights on the host,
giving a combined matrix W [K=640, N=320] where
  K = 512 multivector features (i_ch * 16 + blade) + 128 scalar features
  N = 320 output cols = [mv block (8ch x 16 blades) | 32 s] x 2 selections
Gathers (q_idx / kv_idx item selection) commute with the per-item linear, so
we gather input rows first and compute directly in output order:
  pass A rows = q_idx  -> outputs [q_mv | q_s | v_mv_q | v_s_q]
  pass B rows = kv_idx -> outputs [k_mv | k_s | v_mv   | v_s  ]

Device pipeline per 128-item tile: indirect-DMA row gather (chunked, 1024
rows/op) -> 5 PE transposes (128x128) -> PSUM->SBUF copies -> 5 accumulating
float32r matmuls (K=640 in 5 chunks, N=320) -> bias add -> HWDGE store.
"""

import contextlib
import ctypes
import os
import sys
import types

import numpy as np

import concourse.bacc as bacc
import concourse.bass as bass
import concourse.mybir as mybir
import concourse.tile as tile
from concourse import library_config
from concourse.bass_utils import run_bass_kernel_spmd
from concourse.masks import make_identity
from concourse.tile import add_dep_helper


def _ensure_axon_ntff_hook():
    """bass_utils imports antenv.axon_hooks when trace=True under axon; the
    container ships only a stub antenv package. Inject a working hook module
    backed by direct ctypes calls into libaxon_pjrt.so (same ABI the boot
    shim uses) so NTFF profiling works."""
    try:
        import antenv.axon_hooks  # noqa: F401
        return
    except ImportError:
        pass
    mod = types.ModuleType("antenv.axon_hooks")
    _state = {"hook": None}

    so_path = "/opt/axon/libaxon_pjrt.so"
    if os.path.exists(so_path):
        try:
            lib = ctypes.CDLL(so_path)
            if hasattr(lib, "axon_start_nrt_profile"):
                lib.axon_start_nrt_profile.argtypes = [
                    ctypes.POINTER(ctypes.c_int64), ctypes.c_size_t]
                lib.axon_start_nrt_profile.restype = ctypes.c_int64
                lib.axon_stop_nrt_profile.argtypes = [ctypes.c_char_p]
                lib.axon_stop_nrt_profile.restype = ctypes.c_int64

                @contextlib.contextmanager
                def _hook(output_dir, device_ids):
                    import jax
                    jax.devices()
                    if device_ids:
                        ids = (ctypes.c_int64 * len(device_ids))(*device_ids)
                        rc = lib.axon_start_nrt_profile(ids, len(device_ids))
                    else:
                        rc = lib.axon_start_nrt_profile(None, 0)
                    if rc != 0:
                        raise RuntimeError(f"axon_start_nrt_profile rc={rc}")
                    try:
                        yield
                    finally:
                        n = lib.axon_stop_nrt_profile(str(output_dir).encode())
                        print(f"ntff profile: {n} file(s) -> {output_dir}",
                              file=sys.stderr)

                _state["hook"] = _hook
        except OSError:
            pass

    mod.get_axon_ntff_profile_hook = lambda: _state["hook"]
    mod.set_axon_ntff_profile_hook = lambda h: _state.update(hook=h)
    sys.modules["antenv.axon_hooks"] = mod


_ensure_axon_ntff_hook()

# ---- problem constants (hardcoded; kernel.py must be self-contained)
B, N = 2, 4096
M = B * N                 # 8192 flattened items
C_MV, C_S = 32, 128
NUM_HEADS, HID_MV, HID_S = 8, 8, 32
NF = C_MV * 16 + C_S      # 640 input features per item
NOUT = 320                # output cols per pass: 128 mv + 32 s, x2 selections
P = 128
TILES = M // P            # 64 item-tiles per pass
CHUNK = 8                 # item-tiles gathered per indirect DMA
NCHUNK = TILES // CHUNK
NCORES = 8

_CACHE = {}
LAST_RESULTS = None       # test.py reads this for profiling info


def _build_basis() -> np.ndarray:
    """(9, 16, 16) basis[y, x_out, a_in]: 5 grade projections + 4 e0-left-mult."""
    grades = [[0], [1, 2, 3, 4], [5, 6, 7, 8, 9, 10], [11, 12, 13, 14], [15]]
    maps = []
    for g in grades:
        m = np.zeros((16, 16), np.float32)
        m[g, g] = 1.0
        maps.append(m)
    e0_pairs = [[(1, 0)], [(5, 2), (6, 3), (7, 4)], [(11, 8), (12, 9), (13, 10)],
                [(15, 14)]]
    for pairs in e0_pairs:
        m = np.zeros((16, 16), np.float32)
        for dst, src in pairs:
            m[dst, src] = 1.0
        maps.append(m)
    return np.stack(maps)


_BASIS = _build_basis()


def _fill_block(W, b, col0, w_mv, w_s2mv, w_mv2s, w_s2s, b_s):
    """Fill one 160-col block (128 mv + 32 s) of the combined weight matrix.

    w_mv: (8, C_MV, 9), w_s2mv: (8, C_S), w_mv2s: (32, C_MV), w_s2s: (32, C_S),
    b_s: (32,). Feature rows: f = i*16 + a for mv, 512 + j for scalars.
    mv col = col0 + ch*16 + x; s col = col0 + 128 + cs.
    """
    # out_mv[ch, x] += sum_{i,a,y} w_mv[ch,i,y] * BASIS[y,x,a] * in[i,a]
    wmv = np.einsum("oiy,yxa->iaox", w_mv, _BASIS).reshape(512, 128)
    W[0:512, col0:col0 + 128] = wmv
    # out_mv[ch, 0] += s2mv @ scalars
    W[512:640, col0:col0 + 128:16] = w_s2mv.T
    # out_s[cs] += mv2s @ in[:, 0]  (blade 0 features are rows i*16)
    W[0:512:16, col0 + 128:col0 + 160] = w_mv2s.T
    # out_s[cs] += s2s @ scalars
    W[512:640, col0 + 128:col0 + 160] = w_s2s.T
    b[col0 + 128:col0 + 160] = b_s


def _build_core_weights(h, q_w_mv, q_w_s2mv, q_w_mv2s, q_w_s2s, q_b_s,
                        kv_w_mv, kv_w_s2mv, kv_w_mv2s, kv_w_s2s, kv_b_s):
    """Combined per-head weights: W [2, 640, 320], bias [2, 320]."""
    W = np.zeros((2, NF, NOUT), np.float32)
    bvec = np.zeros((2, NOUT), np.float32)
    # pass A: q block (cols 0:160) + v block (cols 160:320)
    _fill_block(W[0], bvec[0], 0,
                q_w_mv[h::8], q_w_s2mv[h::8], q_w_mv2s[h::8], q_w_s2s[h::8],
                q_b_s[h::8])
    _fill_block(W[0], bvec[0], 160,
                kv_w_mv[64 + h::8], kv_w_s2mv[64 + h::8],
                kv_w_mv2s[256 + h::8], kv_w_s2s[256 + h::8], kv_b_s[256 + h::8])
    # pass B: k block (cols 0:160) + v block (cols 160:320)
    _fill_block(W[1], bvec[1], 0,
                kv_w_mv[h:64:8], kv_w_s2mv[h:64:8],
                kv_w_mv2s[h:256:8], kv_w_s2s[h:256:8], kv_b_s[h:256:8])
    _fill_block(W[1], bvec[1], 160,
                kv_w_mv[64 + h::8], kv_w_s2mv[64 + h::8],
                kv_w_mv2s[256 + h::8], kv_w_s2s[256 + h::8], kv_b_s[256 + h::8])
    return W, bvec


def build_module():
    """Build + compile the SPMD Bass module (same program on all 8 cores)."""
    if "nc" in _CACHE:
        return _CACHE["nc"]
    nc = bacc.Bacc("TRN2", target_bir_lowering=False, debug=False,
                   num_devices=NCORES)
    f32 = mybir.dt.float32
    f16 = mybir.dt.float16
    # rows batch-interleaved: x16[n] = [batch0 feats (640) | batch1 feats]
    x_d = nc.dram_tensor("x16", [N, 2 * NF], f16, kind="ExternalInput")
    w_d = nc.dram_tensor("w", [2, NF, NOUT], f16, kind="ExternalInput")
    b_d = nc.dram_tensor("bias", [2, P, NOUT], f32, kind="ExternalInput")
    NIDX = CHUNK * P // 2          # indices per gather (each serves 2 batches)
    i_d = nc.dram_tensor("idx", [2, NCHUNK, P, NIDX // 16], mybir.dt.int16,
                         kind="ExternalInput")
    o_d = nc.dram_tensor("out", [2, M, NOUT], f32, kind="ExternalOutput")

    JT = CHUNK // 2                # 128-item j-tiles per chunk per batch
    with tile.TileContext(nc) as tc:
        with (
            tc.tile_pool(name="singles", bufs=1) as singles,
            tc.tile_pool(name="chunks", bufs=4) as chunks,
            tc.tile_pool(name="outs", bufs=4) as outs,
            tc.tile_pool(name="psumO", bufs=6, space="PSUM") as psumO_pool,
        ):
            lib = nc.gpsimd.load_library(library_config.mlp)
            w_sb = singles.tile([P, 2, 5, NOUT], f16)
            nc.sync.dma_start(
                out=w_sb, in_=w_d.ap().rearrange("s (c p) n -> p s c n", p=P))
            bias_sb = singles.tile([P, 2, NOUT], f32)
            nc.sync.dma_start(
                out=bias_sb, in_=b_d.ap().rearrange("s p n -> p s n"))
            idx_sb = singles.tile([P, 2, NCHUNK, NIDX // 16], mybir.dt.int16)
            nc.sync.dma_start(
                out=idx_sb, in_=i_d.ap().rearrange("s c p t -> p s c t"))

            for sidx in range(2):
                for c in range(NCHUNK):
                    # transposed gather; chunk kc of xT holds batch kc//5,
                    # features (kc%5)*128 + p
                    xT = chunks.tile([P, 10, NIDX], f16, tag="xT")
                    g1 = nc.gpsimd.dma_gather(
                        xT[:, :, :], x_d.ap(), idx_sb[:, sidx, c, :],
                        NIDX, NIDX, 2 * NF, transpose=True,
                        single_packet=False)
                    add_dep_helper(g1.ins, lib.ins, reason="lib before gather")
                    for b in range(2):
                        for jt in range(JT):
                            po = psumO_pool.tile([P, NOUT], f32)
                            for kc in range(5):
                                nc.tensor.matmul(
                                    out=po[:, :],
                                    lhsT=xT[:, b * 5 + kc,
                                            jt * P:(jt + 1) * P],
                                    rhs=w_sb[:, sidx, kc, :],
                                    start=(kc == 0), stop=(kc == 4))
                            ob = outs.tile([P, NOUT], f32)
                            nc.vector.tensor_add(
                                out=ob[:, :], in0=po[:, :],
                                in1=bias_sb[:, sidx, :])
                            row0 = b * N + (c * JT + jt) * P
                            nc.scalar.dma_start(
                                out=o_d.ap()[sidx, row0:row0 + P, :],
                                in_=ob[:, :])
    nc.compile()
    _CACHE["nc"] = nc
    return nc


def _prep_inputs(inputs, scalars, q_idx, kv_idx, wargs):
    """Host-side shard prep: returns per-core in_maps."""
    # batch-interleaved fp16 rows: x16[n] = [b0 inputs|b0 scalars|b1 ...]
    x16 = np.empty((N, 2, NF), np.float16)
    x16[:, :, :512] = np.asarray(inputs, dtype=np.float32).transpose(
        1, 0, 2, 3).reshape(N, 2, 512)
    x16[:, :, 512:] = np.asarray(scalars, dtype=np.float32).transpose(
        1, 0, 2).reshape(N, 2, 128)
    x16 = np.ascontiguousarray(x16.reshape(N, 2 * NF))

    q_idx = np.asarray(q_idx).astype(np.int64)
    kv_idx = np.asarray(kv_idx).astype(np.int64)
    pidx = np.stack([q_idx, kv_idx])  # [2, N] row indices (both batches)
    # dma_gather idx layout per chunk: index j = s*16 + p sits at [p, s],
    # replicated across the 8 16-partition groups (one per Q7 core)
    NIDX = CHUNK * P // 2
    idx_dev = np.empty((2, NCHUNK, P, NIDX // 16), np.int16)
    for sidx in range(2):
        for c in range(NCHUNK):
            flat = pidx[sidx, c * NIDX:(c + 1) * NIDX].astype(np.int16)
            idx_dev[sidx, c] = np.tile(flat.reshape(NIDX // 16, 16).T, (8, 1))
    idx_dev = np.ascontiguousarray(idx_dev)

    in_maps = []
    for h in range(NCORES):
        W, bvec = _build_core_weights(h, *wargs)
        bias_bcast = np.ascontiguousarray(
            np.broadcast_to(bvec[:, None, :], (2, P, NOUT))).astype(np.float32)
        in_maps.append({
            "x16": x16,
            "w": np.ascontiguousarray(W.astype(np.float16)),
            "bias": bias_bcast, "idx": idx_dev,
        })
    return in_maps


def kernel(inputs, scalars, q_w_mv, q_w_s2mv, q_w_mv2s, q_w_s2s, q_b_s,
           kv_w_mv, kv_w_s2mv, kv_w_mv2s, kv_w_s2s, kv_b_s, q_idx, kv_idx):
    global LAST_RESULTS
    nc = build_module()
    wargs = tuple(np.asarray(a, dtype=np.float32) for a in (
        q_w_mv, q_w_s2mv, q_w_mv2s, q_w_s2s, q_b_s,
        kv_w_mv, kv_w_s2mv, kv_w_mv2s, kv_w_s2s, kv_b_s))
    in_maps = _prep_inputs(inputs, scalars, q_idx, kv_idx, wargs)
    res = run_bass_kernel_spmd(nc, in_maps, core_ids=list(range(NCORES)))
    LAST_RESULTS = res
    o = np.stack([r["out"] for r in res.results])  # [8, 2, M, 320]
    A, Bp = o[:, 0], o[:, 1]

    def mv(block, c0):
        return np.ascontiguousarray(
            block[:, :, c0:c0 + 128].reshape(NCORES, B, N, HID_MV, 16)
            .transpose(1, 0, 2, 3, 4))

    def sc(block, c0):
        return np.ascontiguousarray(
            block[:, :, c0:c0 + 32].reshape(NCORES, B, N, HID_S)
            .transpose(1, 0, 2, 3))

    q_mv, q_s = mv(A, 0), sc(A, 128)
    v_mv_queries, v_s_queries = mv(A, 160), sc(A, 288)
    k_mv, k_s = mv(Bp, 0), sc(Bp, 128)
    v_mv, v_s = mv(Bp, 160), sc(Bp, 288)
    return (q_mv, k_mv, v_mv, v_mv_queries, q_s, k_s, v_s, v_s_queries)
